# revision 9
# baseline (speedup 1.0000x reference)
"""Trainium2 Bass kernel for DSEdgeReadout (edge MLP -> scatter-mean over
graph of source node -> post-pool MLP), SPMD across 8 NeuronCores.

Sharding: edges are sharded across the 8 cores; weights and the (tiny)
graph-boundary vector are replicated.  Each core computes a partial
[num_graphs, hidden+1] matrix of segment sums+counts which is AllReduced
on-device before the mean + post-pool MLP.

Key algorithmic trick: `batch` is sorted, so the per-edge graph id
g_e = batch[src_e] satisfies (g_e >= g) <=> (src_e >= B[g]) where
B[g] = searchsorted(batch, g).  The per-edge one-hot scatter therefore
becomes a dense 0/1 "ge" matrix, built with one vector compare per tile,
and the segment sum becomes a PE matmul accumulated in PSUM:
    S[g, :] = sum_e [src_e >= B[g]] * [1 | h_e]       (cumulative in g)
The true per-graph sums are recovered with a bidiagonal difference
matmul D = M @ S after the AllReduce.
"""

import math
import os
import sys

import numpy as np

_TRN_REPO = "/opt/trn_rl_repo"


def _ensure_path():
    if _TRN_REPO not in sys.path:
        sys.path.insert(0, _TRN_REPO)


_NC_CACHE = {}
LAST_RESULTS = None  # BassKernelResults of the most recent run (for profiling)

TE = 128      # edges per tile (PE contraction dim)
GROUP = 4     # tiles per relu/psum group ([128, 512] = one PSUM bank)
CHUNK = 56    # tiles per DMA chunk (3.67 MB per chunk DMA)
N_HBUF = 4    # SBUF h-tile buffers (fixed tiles w/ preset ones columns)
N_CORES = 8
NUM_GRAPHS = 64


def _build_nc(E_shard, IN, H, OUT, G, b1_nonzero, use_bf16=True):
    _ensure_path()
    import concourse.mybir as mybir
    from concourse import bacc
    from concourse.tile import TileContext

    f32 = mybir.dt.float32
    # PE-facing dtype: fp32 matmuls run at 4 cycles/row on TRN2 (two
    # half-speed passes); bf16 runs at 1.  HBM traffic stays fp32 — the
    # gpsimd (SWDGE) DMA casts inline while loading.
    bdt = mybir.dt.bfloat16 if use_bf16 else f32
    tiles_pc = E_shard // TE

    nc = bacc.Bacc(num_devices=N_CORES, name="ds_edge_readout")

    attrT_d = nc.declare_dram_parameter("attrT", [IN, E_shard], f32, isOutput=False)
    src2d_d = nc.declare_dram_parameter("src2d", [TE, tiles_pc], f32, isOutput=False)
    Brep_d = nc.declare_dram_parameter("Brep", [TE, G], f32, isOutput=False)
    W1_d = nc.declare_dram_parameter("W1", [IN, H], f32, isOutput=False)
    W2_d = nc.declare_dram_parameter("W2", [H, H], f32, isOutput=False)
    W3_d = nc.declare_dram_parameter("W3", [H, OUT], f32, isOutput=False)
    b2c_d = nc.declare_dram_parameter("b2c", [H, 1], f32, isOutput=False)
    b3c_d = nc.declare_dram_parameter("b3c", [OUT, 1], f32, isOutput=False)
    MT_d = nc.declare_dram_parameter("MT", [G, G], f32, isOutput=False)
    IDG_d = nc.declare_dram_parameter("IDG", [G, G], f32, isOutput=False)
    if b1_nonzero:
        b1rep_d = nc.declare_dram_parameter("b1rep", [TE, GROUP * H], f32, isOutput=False)
    outT_d = nc.declare_dram_parameter("outT", [OUT, G], f32, isOutput=True)

    # collective bounce buffers (cannot be I/O tensors)
    cc_in = nc.dram_tensor("cc_in", [G, H + 1], f32)
    cc_out = nc.dram_tensor("cc_out", [G, H + 1], f32, addr_space="Shared")

    Relu = mybir.ActivationFunctionType.Relu
    Identity = mybir.ActivationFunctionType.Identity
    Alu = mybir.AluOpType

    with TileContext(nc) as tc:
        with (
            tc.tile_pool(name="const", bufs=1) as cpool,
            tc.tile_pool(name="attr", bufs=3) as apool,
            tc.tile_pool(name="src", bufs=3) as spool,
            tc.tile_pool(name="ge", bufs=2 * GROUP + 2) as gpool,
            tc.tile_pool(name="hbuf", bufs=N_HBUF) as hpool,
            tc.tile_pool(name="ph", bufs=3, space="PSUM") as php,
            tc.tile_pool(name="pmisc", bufs=1, space="PSUM") as pmp,
        ):
            # ---- constants ----
            W1_sb = cpool.tile([IN, H], bdt, tag="w1")
            if use_bf16:
                nc.gpsimd.dma_start(out=W1_sb[:, :], in_=W1_d[:, :])
            else:
                nc.sync.dma_start(out=W1_sb[:, :], in_=W1_d[:, :])
            Brep_sb = cpool.tile([TE, G], f32, tag="brep")
            nc.sync.dma_start(out=Brep_sb[:, :], in_=Brep_d[:, :])
            W2_sb = cpool.tile([H, H], f32, tag="w2")
            nc.sync.dma_start(out=W2_sb[:, :], in_=W2_d[:, :])
            W3_sb = cpool.tile([H, OUT], f32, tag="w3")
            nc.sync.dma_start(out=W3_sb[:, :], in_=W3_d[:, :])
            b2c_sb = cpool.tile([H, 1], f32, tag="b2c")
            nc.sync.dma_start(out=b2c_sb[:, :], in_=b2c_d[:, :])
            b3c_sb = cpool.tile([OUT, 1], f32, tag="b3c")
            nc.sync.dma_start(out=b3c_sb[:, :], in_=b3c_d[:, :])
            MT_sb = cpool.tile([G, G], f32, tag="mt")
            nc.sync.dma_start(out=MT_sb[:, :], in_=MT_d[:, :])
            IDG_sb = cpool.tile([G, G], f32, tag="idg")
            nc.sync.dma_start(out=IDG_sb[:, :], in_=IDG_d[:, :])
            if b1_nonzero:
                b1rep_sb = cpool.tile([TE, GROUP * H], f32, tag="b1rep")
                nc.sync.dma_start(out=b1rep_sb[:, :], in_=b1rep_d[:, :])

            # fixed h tiles: [ones | h0 | ones | h1 | ...], ones preset once
            h_tiles = []
            for i in range(N_HBUF):
                ht = hpool.tile([TE, GROUP * (H + 1)], bdt, tag="h", name=f"ht{i}")
                for j in range(GROUP):
                    nc.vector.memset(ht[:, j * (H + 1) : j * (H + 1) + 1], 1.0)
                h_tiles.append(ht)

            # PSUM accumulator for cumulative segment sums (+counts col 0)
            S_ps = pmp.tile([G, H + 1], f32, tag="S")

            n_tiles = tiles_pc
            pending = None  # (h_tile, [ge...], gsz, base_tile_idx)

            def emit_mm2(p):
                ht, ges, gsz, base = p
                for j in range(gsz):
                    tg = base + j
                    nc.tensor.matmul(
                        S_ps[:, :],
                        lhsT=ges[j][:, :],
                        rhs=ht[:, j * (H + 1) : (j + 1) * (H + 1)],
                        start=(tg == 0),
                        stop=(tg == n_tiles - 1),
                    )

            hb = 0
            t0 = 0
            first = True
            while t0 < n_tiles:
                # split the first chunk into small pieces so compute starts
                # after ~1 tile-DMA instead of a full 3.7MB chunk DMA
                ct = min(8 if first else CHUNK, n_tiles - t0)
                first = False
                at = apool.tile([TE, CHUNK * TE], bdt, tag="attr")
                if use_bf16:
                    nc.gpsimd.dma_start(
                        out=at[:, : ct * TE],
                        in_=attrT_d[:, t0 * TE : (t0 + ct) * TE],
                    )
                else:
                    nc.sync.dma_start(
                        out=at[:, : ct * TE],
                        in_=attrT_d[:, t0 * TE : (t0 + ct) * TE],
                    )
                st = spool.tile([TE, CHUNK], f32, tag="src")
                nc.sync.dma_start(out=st[:, :ct], in_=src2d_d[:, t0 : t0 + ct])

                for g0 in range(0, ct, GROUP):
                    gsz = min(GROUP, ct - g0)
                    ph = php.tile([TE, GROUP * H], f32, tag="ph")
                    ht = h_tiles[hb]
                    hb = (hb + 1) % N_HBUF
                    for j in range(gsz):
                        t = g0 + j
                        nc.tensor.matmul(
                            ph[:, j * H : (j + 1) * H],
                            lhsT=at[:, t * TE : (t + 1) * TE],
                            rhs=W1_sb[:, :],
                            start=True,
                            stop=True,
                        )
                    if b1_nonzero:
                        nc.vector.tensor_tensor(
                            ph[:, : gsz * H],
                            ph[:, : gsz * H],
                            b1rep_sb[:, : gsz * H],
                            Alu.add,
                        )
                    # relu: PSUM [128, gsz*128] -> SBUF strided (skip ones cols)
                    in_ap = ph[:, : gsz * H].rearrange("p (t c) -> p t c", c=H)
                    out_ap = ht[:, : gsz * (H + 1)].rearrange(
                        "p (t c) -> p t c", c=H + 1
                    )[:, :, 1:]
                    nc.scalar.activation(out_ap, in_ap, Relu)
                    # ge tiles (DVE): ge[p, g] = (B[g] <= src_p)
                    ges = []
                    for j in range(gsz):
                        t = g0 + j
                        ge = gpool.tile([TE, G], bdt, tag="ge")
                        nc.vector.tensor_scalar(
                            ge[:, :],
                            Brep_sb[:, :],
                            st[:, t : t + 1],
                            None,
                            Alu.is_le,
                        )
                        ges.append(ge)
                    # software skew: emit previous group's segment matmuls now
                    if pending is not None:
                        emit_mm2(pending)
                    pending = (ht, ges, gsz, t0 + g0)
                t0 += ct

            emit_mm2(pending)

            # ---- epilogue: allreduce, diff, mean, post-pool MLP ----
            S_sb = cpool.tile([G, H + 1], f32, tag="ssb")
            nc.vector.tensor_copy(S_sb[:, :], S_ps[:, :])
            nc.sync.dma_start(out=cc_in[:, :], in_=S_sb[:, :])
            nc.gpsimd.collective_compute(
                "AllReduce",
                Alu.add,
                replica_groups=[list(range(N_CORES))],
                ins=[cc_in[:, :].opt()],
                outs=[cc_out[:, :].opt()],
            )
            Sall = cpool.tile([G, H + 1], f32, tag="sall")
            nc.sync.dma_start(out=Sall[:, :], in_=cc_out[:, :])

            # D[g,:] = S[g,:] - S[g+1,:]  via bidiagonal matmul
            D_ps = pmp.tile([G, H + 1], f32, tag="D")
            nc.tensor.matmul(D_ps[:, :], lhsT=MT_sb[:, :], rhs=Sall[:, :],
                             start=True, stop=True)
            cnt = cpool.tile([G, 1], f32, tag="cnt")
            nc.vector.tensor_scalar(cnt[:, :], D_ps[:, 0:1], 1.0, None, Alu.max)
            rec = cpool.tile([G, 1], f32, tag="rec")
            nc.vector.reciprocal(rec[:, :], cnt[:, :])
            gf = cpool.tile([G, H], f32, tag="gf")
            nc.vector.tensor_scalar(
                gf[:, :], D_ps[:, 1 : H + 1], rec[:, 0:1], None, Alu.mult
            )
            # transpose gf -> [H, G]
            gfT_ps = pmp.tile([H, G], f32, tag="gft")
            nc.tensor.transpose(gfT_ps[:, :], gf[:, :], IDG_sb[:, :])
            gfT = cpool.tile([H, G], f32, tag="gftsb")
            nc.scalar.copy(gfT[:, :], gfT_ps[:, :])
            # z2T = W2.T-contract: [h2, g]
            z2_ps = pmp.tile([H, G], f32, tag="z2")
            nc.tensor.matmul(z2_ps[:, :], lhsT=W2_sb[:, :], rhs=gfT[:, :],
                             start=True, stop=True)
            a2 = cpool.tile([H, G], f32, tag="a2")
            nc.scalar.activation(a2[:, :], z2_ps[:, :], Relu, bias=b2c_sb[:, 0:1])
            o_ps = pmp.tile([OUT, G], f32, tag="ops")
            nc.tensor.matmul(o_ps[:, :], lhsT=W3_sb[:, :], rhs=a2[:, :],
                             start=True, stop=True)
            o_sb = cpool.tile([OUT, G], f32, tag="osb")
            nc.scalar.activation(o_sb[:, :], o_ps[:, :], Identity,
                                 bias=b3c_sb[:, 0:1])
            nc.sync.dma_start(out=outT_d[:, :], in_=o_sb[:, :])

    nc.compile()
    return nc


def prepare(edge_index, edge_attr, batch, W1, b1, W2, b2, W3, b3):
    """Builds (or fetches cached) the Bass program and the per-core input
    maps for the given full inputs.  Host work is layout-only: shard +
    transpose edge_attr, reshape src ids, searchsorted boundaries."""
    _ensure_path()
    edge_index = np.asarray(edge_index)
    edge_attr = np.asarray(edge_attr, dtype=np.float32)
    batch = np.asarray(batch)
    W1 = np.asarray(W1, dtype=np.float32)
    b1 = np.asarray(b1, dtype=np.float32)
    W2 = np.asarray(W2, dtype=np.float32)
    b2 = np.asarray(b2, dtype=np.float32)
    W3 = np.asarray(W3, dtype=np.float32)
    b3 = np.asarray(b3, dtype=np.float32)

    E, IN = edge_attr.shape
    H = W1.shape[1]
    OUT = W3.shape[1]
    G = NUM_GRAPHS

    tiles_pc = math.ceil(E / (TE * N_CORES))
    E_shard = tiles_pc * TE

    srcf = edge_index[0].astype(np.float32)

    b1_nonzero = bool(np.any(b1))
    key = (E_shard, IN, H, OUT, G, b1_nonzero)
    nc = _NC_CACHE.get(key)
    if nc is None:
        nc = _build_nc(E_shard, IN, H, OUT, G, b1_nonzero)
        _NC_CACHE[key] = nc

    B = np.searchsorted(batch, np.arange(G), side="left").astype(np.float32)
    Brep = np.ascontiguousarray(np.broadcast_to(B, (TE, G)))
    MT = np.eye(G, dtype=np.float32)
    MT[np.arange(1, G), np.arange(0, G - 1)] = -1.0
    IDG = np.eye(G, dtype=np.float32)
    b2c = np.ascontiguousarray(b2.reshape(H, 1))
    b3c = np.ascontiguousarray(b3.reshape(OUT, 1))

    in_maps = []
    for c in range(N_CORES):
        lo = c * E_shard
        hi = min(E, lo + E_shard)
        n = max(0, hi - lo)
        at = np.zeros((IN, E_shard), dtype=np.float32)
        if n:
            at[:, :n] = edge_attr[lo:hi].T
        sv = np.full((E_shard,), -1.0, dtype=np.float32)
        if n:
            sv[:n] = srcf[lo:hi]
        s2 = np.ascontiguousarray(sv.reshape(tiles_pc, TE).T)
        m = dict(
            attrT=at, src2d=s2, Brep=Brep, W1=W1, W2=W2, W3=W3,
            b2c=b2c, b3c=b3c, MT=MT, IDG=IDG,
        )
        if b1_nonzero:
            m["b1rep"] = np.ascontiguousarray(np.tile(b1, (TE, GROUP)))
        in_maps.append(m)

    return nc, in_maps


def kernel(edge_index, edge_attr, batch, W1, b1, W2, b2, W3, b3):
    global LAST_RESULTS
    _ensure_path()
    from concourse.bass_utils import run_bass_kernel_spmd

    nc, in_maps = prepare(edge_index, edge_attr, batch, W1, b1, W2, b2, W3, b3)
    trace = bool(int(os.environ.get("DSER_TRACE", "0")))
    res = None
    last_exc = None
    for _attempt in range(3):  # retry transient device wedges
        try:
            res = run_bass_kernel_spmd(nc, in_maps, list(range(N_CORES)),
                                       trace=trace)
            break
        except Exception as e:  # noqa: BLE001
            last_exc = e
    if res is None:
        raise last_exc
    LAST_RESULTS = res
    out = np.ascontiguousarray(res.results[0]["outT"].T).astype(np.float32)
    return out


# revision 12
# speedup vs baseline: 1.3021x; 1.3021x over previous
"""Trainium2 Bass kernel for DSEdgeReadout (edge MLP -> scatter-mean over
graph of source node -> post-pool MLP), SPMD across 8 NeuronCores.

Sharding: edges are sharded across the 8 cores; weights and the (tiny)
graph-boundary vector are replicated.  Each core computes a partial
[num_graphs, hidden+1] matrix of segment sums+counts which is AllReduced
on-device before the mean + post-pool MLP.

Key algorithmic trick: `batch` is sorted, so the per-edge graph id
g_e = batch[src_e] satisfies (g_e >= g) <=> (src_e >= B[g]) where
B[g] = searchsorted(batch, g).  The per-edge one-hot scatter therefore
becomes a dense 0/1 "ge" matrix, built with one vector compare per tile,
and the segment sum becomes a PE matmul accumulated in PSUM:
    S[g, :] = sum_e [src_e >= B[g]] * [1 | h_e]       (cumulative in g)
The true per-graph sums are recovered with a bidiagonal difference
matmul D = M @ S after the AllReduce.
"""

import math
import os
import sys

import numpy as np

_TRN_REPO = "/opt/trn_rl_repo"


def _ensure_path():
    if _TRN_REPO not in sys.path:
        sys.path.insert(0, _TRN_REPO)


_NC_CACHE = {}
LAST_RESULTS = None  # BassKernelResults of the most recent run (for profiling)

TE = 128      # edges per tile (PE contraction dim)
GROUP = 4     # tiles per relu/psum group ([128, 512] = one PSUM bank)
CHUNK = 56    # tiles per DMA chunk (3.67 MB per chunk DMA)
N_HBUF = 4    # SBUF h-tile buffers (fixed tiles w/ preset ones columns)
N_CORES = 8
NUM_GRAPHS = 64


def _build_nc(E_shard, IN, H, OUT, G, b1_nonzero, use_bf16=True):
    _ensure_path()
    import concourse.mybir as mybir
    from concourse import bacc
    from concourse.tile import TileContext

    f32 = mybir.dt.float32
    # PE-facing dtype: fp32 matmuls run at 4 cycles/row on TRN2 (two
    # half-speed passes); bf16 runs at 1.  HBM traffic stays fp32 — the
    # gpsimd (SWDGE) DMA casts inline while loading.
    bdt = mybir.dt.bfloat16 if use_bf16 else f32
    tiles_pc = E_shard // TE

    nc = bacc.Bacc(num_devices=N_CORES, name="ds_edge_readout")

    attrT_d = nc.declare_dram_parameter("attrT", [IN, E_shard], f32, isOutput=False)
    src2d_d = nc.declare_dram_parameter("src2d", [TE, tiles_pc], f32, isOutput=False)
    Brep_d = nc.declare_dram_parameter("Brep", [TE, G], f32, isOutput=False)
    W1_d = nc.declare_dram_parameter("W1", [IN, H], f32, isOutput=False)
    W2_d = nc.declare_dram_parameter("W2", [H, H], f32, isOutput=False)
    W3_d = nc.declare_dram_parameter("W3", [H, OUT], f32, isOutput=False)
    b2c_d = nc.declare_dram_parameter("b2c", [H, 1], f32, isOutput=False)
    b3c_d = nc.declare_dram_parameter("b3c", [OUT, 1], f32, isOutput=False)
    MT_d = nc.declare_dram_parameter("MT", [G, G], f32, isOutput=False)
    IDG_d = nc.declare_dram_parameter("IDG", [G, G], f32, isOutput=False)
    if b1_nonzero:
        b1rep_d = nc.declare_dram_parameter("b1rep", [TE, GROUP * H], f32, isOutput=False)
    outT_d = nc.declare_dram_parameter("outT", [OUT, G], f32, isOutput=True)

    # collective bounce buffers (cannot be I/O tensors)
    cc_in = nc.dram_tensor("cc_in", [G, H + 1], f32)
    cc_out = nc.dram_tensor("cc_out", [N_CORES * G, H + 1], f32,
                            addr_space="Shared")

    Relu = mybir.ActivationFunctionType.Relu
    Identity = mybir.ActivationFunctionType.Identity
    Alu = mybir.AluOpType

    with TileContext(nc) as tc:
        with (
            tc.tile_pool(name="const", bufs=1) as cpool,
            tc.tile_pool(name="attr", bufs=3) as apool,
            tc.tile_pool(name="src", bufs=3) as spool,
            tc.tile_pool(name="ge", bufs=2 * GROUP + 2) as gpool,
            tc.tile_pool(name="hbuf", bufs=N_HBUF) as hpool,
            tc.tile_pool(name="ph", bufs=3, space="PSUM") as php,
            tc.tile_pool(name="pmisc", bufs=1, space="PSUM") as pmp,
        ):
            # ---- constants ----
            W1_sb = cpool.tile([IN, H], bdt, tag="w1")
            if use_bf16:
                nc.gpsimd.dma_start(out=W1_sb[:, :], in_=W1_d[:, :])
            else:
                nc.sync.dma_start(out=W1_sb[:, :], in_=W1_d[:, :])
            Brep_sb = cpool.tile([TE, G], f32, tag="brep")
            nc.sync.dma_start(out=Brep_sb[:, :], in_=Brep_d[:, :])
            W2_sb = cpool.tile([H, H], f32, tag="w2")
            nc.sync.dma_start(out=W2_sb[:, :], in_=W2_d[:, :])
            W3_sb = cpool.tile([H, OUT], f32, tag="w3")
            nc.sync.dma_start(out=W3_sb[:, :], in_=W3_d[:, :])
            b2c_sb = cpool.tile([H, 1], f32, tag="b2c")
            nc.sync.dma_start(out=b2c_sb[:, :], in_=b2c_d[:, :])
            b3c_sb = cpool.tile([OUT, 1], f32, tag="b3c")
            nc.sync.dma_start(out=b3c_sb[:, :], in_=b3c_d[:, :])
            MT_sb = cpool.tile([G, G], f32, tag="mt")
            nc.sync.dma_start(out=MT_sb[:, :], in_=MT_d[:, :])
            IDG_sb = cpool.tile([G, G], f32, tag="idg")
            nc.sync.dma_start(out=IDG_sb[:, :], in_=IDG_d[:, :])
            if b1_nonzero:
                b1rep_sb = cpool.tile([TE, GROUP * H], f32, tag="b1rep")
                nc.sync.dma_start(out=b1rep_sb[:, :], in_=b1rep_d[:, :])

            # fixed h tiles: [ones | h0 | ones | h1 | ...], ones preset once
            h_tiles = []
            for i in range(N_HBUF):
                ht = hpool.tile([TE, GROUP * (H + 1)], bdt, tag="h", name=f"ht{i}")
                for j in range(GROUP):
                    nc.vector.memset(ht[:, j * (H + 1) : j * (H + 1) + 1], 1.0)
                h_tiles.append(ht)

            # PSUM accumulator for cumulative segment sums (+counts col 0)
            S_ps = pmp.tile([G, H + 1], f32, tag="S")

            n_tiles = tiles_pc
            pending = None  # (h_tile, [ge...], gsz, base_tile_idx)

            def emit_mm2(p):
                ht, ges, gsz, base = p
                for j in range(gsz):
                    tg = base + j
                    nc.tensor.matmul(
                        S_ps[:, :],
                        lhsT=ges[j][:, :],
                        rhs=ht[:, j * (H + 1) : (j + 1) * (H + 1)],
                        start=(tg == 0),
                        stop=(tg == n_tiles - 1),
                    )

            hb = 0
            grp_i = 0
            t0 = 0
            first = True
            while t0 < n_tiles:
                # split the first chunk into small pieces so compute starts
                # after ~1 tile-DMA instead of a full 3.7MB chunk DMA
                ct = min(8 if first else CHUNK, n_tiles - t0)
                first = False
                at = apool.tile([TE, CHUNK * TE], bdt, tag="attr")
                if use_bf16:
                    nc.gpsimd.dma_start(
                        out=at[:, : ct * TE],
                        in_=attrT_d[:, t0 * TE : (t0 + ct) * TE],
                    )
                else:
                    nc.sync.dma_start(
                        out=at[:, : ct * TE],
                        in_=attrT_d[:, t0 * TE : (t0 + ct) * TE],
                    )
                st = spool.tile([TE, CHUNK], f32, tag="src")
                nc.sync.dma_start(out=st[:, :ct], in_=src2d_d[:, t0 : t0 + ct])

                for g0 in range(0, ct, GROUP):
                    gsz = min(GROUP, ct - g0)
                    ph = php.tile([TE, GROUP * H], f32, tag="ph")
                    ht = h_tiles[hb]
                    hb = (hb + 1) % N_HBUF
                    for j in range(gsz):
                        t = g0 + j
                        nc.tensor.matmul(
                            ph[:, j * H : (j + 1) * H],
                            lhsT=at[:, t * TE : (t + 1) * TE],
                            rhs=W1_sb[:, :],
                            start=True,
                            stop=True,
                        )
                    if b1_nonzero:
                        nc.vector.tensor_tensor(
                            ph[:, : gsz * H],
                            ph[:, : gsz * H],
                            b1rep_sb[:, : gsz * H],
                            Alu.add,
                        )
                    # relu: PSUM [128, gsz*128] -> SBUF strided (skip ones cols)
                    in_ap = ph[:, : gsz * H].rearrange("p (t c) -> p t c", c=H)
                    out_ap = ht[:, : gsz * (H + 1)].rearrange(
                        "p (t c) -> p t c", c=H + 1
                    )[:, :, 1:]
                    if grp_i % 5 == 4:
                        nc.vector.tensor_scalar(
                            out_ap, in_ap, 0.0, None, Alu.max)
                    else:
                        nc.scalar.activation(out_ap, in_ap, Relu)
                    grp_i += 1
                    # ge tiles (DVE): ge[p, g] = (B[g] <= src_p)
                    ges = []
                    for j in range(gsz):
                        t = g0 + j
                        ge = gpool.tile([TE, G], bdt, tag="ge")
                        nc.vector.tensor_scalar(
                            ge[:, :],
                            Brep_sb[:, :],
                            st[:, t : t + 1],
                            None,
                            Alu.is_le,
                        )
                        ges.append(ge)
                    # software skew: emit previous group's segment matmuls now
                    if pending is not None:
                        emit_mm2(pending)
                    pending = (ht, ges, gsz, t0 + g0)
                t0 += ct

            emit_mm2(pending)

            # ---- epilogue: allreduce, diff, mean, post-pool MLP ----
            S_sb = cpool.tile([G, H + 1], f32, tag="ssb")
            nc.vector.tensor_copy(S_sb[:, :], S_ps[:, :])
            nc.sync.dma_start(out=cc_in[:, :], in_=S_sb[:, :])
            # AllGather + local reduce: an AllReduce is internally
            # ReduceScatter+AllGather, so gathering the 33KB partials and
            # summing 8 blocks locally (1us of DVE) halves the collective.
            nc.gpsimd.collective_compute(
                "AllGather",
                Alu.bypass,
                replica_groups=[list(range(N_CORES))],
                ins=[cc_in[:, :].opt()],
                outs=[cc_out[:, :].opt()],
            )
            Sg = cpool.tile([G, N_CORES * (H + 1)], f32, tag="sg")
            nc.sync.dma_start(
                out=Sg[:, :].rearrange("p (r c) -> p r c", c=H + 1),
                in_=cc_out[:, :].rearrange("(r p) c -> p r c", r=N_CORES),
            )
            Sall = cpool.tile([G, H + 1], f32, tag="sall")
            nc.vector.tensor_tensor(
                Sall[:, :], Sg[:, 0 : H + 1], Sg[:, H + 1 : 2 * (H + 1)],
                Alu.add,
            )
            for r in range(2, N_CORES):
                nc.vector.tensor_tensor(
                    Sall[:, :], Sall[:, :],
                    Sg[:, r * (H + 1) : (r + 1) * (H + 1)], Alu.add,
                )

            # D[g,:] = S[g,:] - S[g+1,:]  via bidiagonal matmul
            D_ps = pmp.tile([G, H + 1], f32, tag="D")
            nc.tensor.matmul(D_ps[:, :], lhsT=MT_sb[:, :], rhs=Sall[:, :],
                             start=True, stop=True)
            cnt = cpool.tile([G, 1], f32, tag="cnt")
            nc.vector.tensor_scalar(cnt[:, :], D_ps[:, 0:1], 1.0, None, Alu.max)
            rec = cpool.tile([G, 1], f32, tag="rec")
            nc.vector.reciprocal(rec[:, :], cnt[:, :])
            gf = cpool.tile([G, H], f32, tag="gf")
            nc.vector.tensor_scalar(
                gf[:, :], D_ps[:, 1 : H + 1], rec[:, 0:1], None, Alu.mult
            )
            # transpose gf -> [H, G]
            gfT_ps = pmp.tile([H, G], f32, tag="gft")
            nc.tensor.transpose(gfT_ps[:, :], gf[:, :], IDG_sb[:, :])
            gfT = cpool.tile([H, G], f32, tag="gftsb")
            nc.scalar.copy(gfT[:, :], gfT_ps[:, :])
            # z2T = W2.T-contract: [h2, g]
            z2_ps = pmp.tile([H, G], f32, tag="z2")
            nc.tensor.matmul(z2_ps[:, :], lhsT=W2_sb[:, :], rhs=gfT[:, :],
                             start=True, stop=True)
            a2 = cpool.tile([H, G], f32, tag="a2")
            nc.scalar.activation(a2[:, :], z2_ps[:, :], Relu, bias=b2c_sb[:, 0:1])
            o_ps = pmp.tile([OUT, G], f32, tag="ops")
            nc.tensor.matmul(o_ps[:, :], lhsT=W3_sb[:, :], rhs=a2[:, :],
                             start=True, stop=True)
            o_sb = cpool.tile([OUT, G], f32, tag="osb")
            nc.scalar.activation(o_sb[:, :], o_ps[:, :], Identity,
                                 bias=b3c_sb[:, 0:1])
            nc.sync.dma_start(out=outT_d[:, :], in_=o_sb[:, :])

    nc.compile()
    return nc


def prepare(edge_index, edge_attr, batch, W1, b1, W2, b2, W3, b3):
    """Builds (or fetches cached) the Bass program and the per-core input
    maps for the given full inputs.  Host work is layout-only: shard +
    transpose edge_attr, reshape src ids, searchsorted boundaries."""
    _ensure_path()
    edge_index = np.asarray(edge_index)
    edge_attr = np.asarray(edge_attr, dtype=np.float32)
    batch = np.asarray(batch)
    W1 = np.asarray(W1, dtype=np.float32)
    b1 = np.asarray(b1, dtype=np.float32)
    W2 = np.asarray(W2, dtype=np.float32)
    b2 = np.asarray(b2, dtype=np.float32)
    W3 = np.asarray(W3, dtype=np.float32)
    b3 = np.asarray(b3, dtype=np.float32)

    E, IN = edge_attr.shape
    H = W1.shape[1]
    OUT = W3.shape[1]
    G = NUM_GRAPHS

    tiles_pc = math.ceil(E / (TE * N_CORES))
    E_shard = tiles_pc * TE

    srcf = edge_index[0].astype(np.float32)

    b1_nonzero = bool(np.any(b1))
    key = (E_shard, IN, H, OUT, G, b1_nonzero)
    nc = _NC_CACHE.get(key)
    if nc is None:
        nc = _build_nc(E_shard, IN, H, OUT, G, b1_nonzero)
        _NC_CACHE[key] = nc

    B = np.searchsorted(batch, np.arange(G), side="left").astype(np.float32)
    Brep = np.ascontiguousarray(np.broadcast_to(B, (TE, G)))
    MT = np.eye(G, dtype=np.float32)
    MT[np.arange(1, G), np.arange(0, G - 1)] = -1.0
    IDG = np.eye(G, dtype=np.float32)
    b2c = np.ascontiguousarray(b2.reshape(H, 1))
    b3c = np.ascontiguousarray(b3.reshape(OUT, 1))

    in_maps = []
    for c in range(N_CORES):
        lo = c * E_shard
        hi = min(E, lo + E_shard)
        n = max(0, hi - lo)
        at = np.zeros((IN, E_shard), dtype=np.float32)
        if n:
            at[:, :n] = edge_attr[lo:hi].T
        sv = np.full((E_shard,), -1.0, dtype=np.float32)
        if n:
            sv[:n] = srcf[lo:hi]
        s2 = np.ascontiguousarray(sv.reshape(tiles_pc, TE).T)
        m = dict(
            attrT=at, src2d=s2, Brep=Brep, W1=W1, W2=W2, W3=W3,
            b2c=b2c, b3c=b3c, MT=MT, IDG=IDG,
        )
        if b1_nonzero:
            m["b1rep"] = np.ascontiguousarray(np.tile(b1, (TE, GROUP)))
        in_maps.append(m)

    return nc, in_maps


def _kernel_impl(edge_index, edge_attr, batch, W1, b1, W2, b2, W3, b3):
    global LAST_RESULTS
    _ensure_path()
    from concourse.bass_utils import run_bass_kernel_spmd

    nc, in_maps = prepare(edge_index, edge_attr, batch, W1, b1, W2, b2, W3, b3)
    trace = bool(int(os.environ.get("DSER_TRACE", "0")))
    res = run_bass_kernel_spmd(nc, in_maps, list(range(N_CORES)), trace=trace)
    LAST_RESULTS = res
    out = np.ascontiguousarray(res.results[0]["outT"].T).astype(np.float32)
    return out


def _kernel_subprocess(**inputs):
    """Re-run in a fresh process.  A wedged NeuronCore (NRT_EXEC_UNIT_
    UNRECOVERABLE) poisons the whole axon mesh for the current process;
    a fresh process re-attaches cleanly."""
    import subprocess
    import tempfile

    d = tempfile.mkdtemp(prefix="dser_")
    in_path = os.path.join(d, "in.npz")
    out_path = os.path.join(d, "out.npy")
    np.savez(in_path, **inputs)
    here = os.path.dirname(os.path.abspath(__file__))
    code = (
        "import sys; sys.path.insert(0, %r)\n"
        "import numpy as np, kernel\n"
        "d = dict(np.load(%r))\n"
        "out = kernel._kernel_impl(**d)\n"
        "np.save(%r, out)\n" % (here, in_path, out_path)
    )
    subprocess.run([sys.executable, "-c", code], check=True, timeout=3000)
    return np.load(out_path)


def kernel(edge_index, edge_attr, batch, W1, b1, W2, b2, W3, b3):
    inputs = dict(edge_index=edge_index, edge_attr=edge_attr, batch=batch,
                  W1=W1, b1=b1, W2=W2, b2=b2, W3=W3, b3=b3)
    try:
        return _kernel_impl(**inputs)
    except Exception:  # noqa: BLE001
        return _kernel_subprocess(**inputs)
